# Initial kernel scaffold
#
"""Trainium2 Bass kernel for AdvancedNeuralStructureAnalyzer.

Sharding: 8 cores = 2 batches x 4 pair-quarters.
Core c: batch b=c//4 computes full phase A (heads, 3 GNN layers, attention)
redundantly within its batch group; pair-quarter q=c%4 computes relation MLP
for units u = q+4k (mirror-paired i-blocks of uniform 256-pair size).

Device layout: activations feature-major ("T" = [features->partitions, nodes->free]).
All weights host-pre-transposed to [in_features, out_features] so no on-device
weight transposes are needed. Matmuls run as fp32r (bitcast).
"""
import numpy as np

B, N, D, E, HEADS = 2, 256, 768, 8192, 8
HD = D // HEADS            # 96
KT = D // 128              # 6
EPS = 1e-5
P = N * (N - 1) // 2       # 32640
NCORES = 8
SLOTS = 8192               # per-core padded pair slots (32 units x 256)
NTILES = 16                # 512-slot tiles per core

_prog_cache = {}


def _units(q):
    return [q + 4 * k for k in range(32)]


def _segments(q):
    """Per 512-tile: list of (i_col, j0, ln, dst) segments. Uniform formulas:
    unit u at base beta: (u, u+1, 255-u, beta) and (254-u, 255-u, u+1, beta+255-u)."""
    us = _units(q)
    tiles = []
    for t in range(NTILES):
        segs = []
        for s, u in enumerate((us[2 * t], us[2 * t + 1])):
            beta = 256 * s
            segs.append((u, u + 1, 255 - u, beta))
            segs.append((254 - u, 255 - u, u + 1, beta + 255 - u))
        tiles.append(segs)
    return tiles


def _build_program():
    import concourse.bass as bass
    import concourse.tile as tile
    from concourse import mybir
    from concourse.masks import make_identity

    F32 = mybir.dt.float32
    F32R = mybir.dt.float32r
    AF = mybir.ActivationFunctionType
    OP = mybir.AluOpType

    def R(ap):
        return ap.bitcast(F32R)

    nc = bass.Bass()
    dp = nc.declare_dram_parameter
    # per-core inputs
    xT = dp("xT", [D, N], F32, isOutput=False)
    ST = dp("ST", [N, N], F32, isOutput=False)
    # weights (shared values across cores), all pre-transposed [in, out]
    oc_w1T = dp("oc_w1T", [D, D], F32, isOutput=False)
    oc_b1 = dp("oc_b1", [D, 1], F32, isOutput=False)
    oc_w2T = dp("oc_w2T", [D, 384], F32, isOutput=False)
    oc_b2 = dp("oc_b2", [384, 1], F32, isOutput=False)
    oc_w3T = dp("oc_w3T", [384, 8], F32, isOutput=False)
    oc_b3r = dp("oc_b3r", [1, 8], F32, isOutput=False)
    ip_w1T = dp("ip_w1T", [D, 384], F32, isOutput=False)
    ip_b1 = dp("ip_b1", [384, 1], F32, isOutput=False)
    ip_w2T = dp("ip_w2T", [384, 192], F32, isOutput=False)
    ip_b2 = dp("ip_b2", [192, 1], F32, isOutput=False)
    ip_w3T = dp("ip_w3T", [192, 1], F32, isOutput=False)
    ip_b3r = dp("ip_b3r", [1, 1], F32, isOutput=False)
    gnn_w = [dp(f"gnn{l}_w", [D, 2 * D], F32, isOutput=False) for l in range(3)]
    gnn_bsn = [dp(f"gnn{l}_bsn", [D, 1], F32, isOutput=False) for l in range(3)]
    gnn_g = [dp(f"gnn{l}_gm", [D, 1], F32, isOutput=False) for l in range(3)]
    gnn_bt = [dp(f"gnn{l}_bt", [D, 1], F32, isOutput=False) for l in range(3)]
    attn_inT = dp("attn_inT", [D, 3 * D], F32, isOutput=False)
    attn_inbr = dp("attn_inbr", [1, 3 * D], F32, isOutput=False)
    attn_outT = dp("attn_outT", [D, D], F32, isOutput=False)
    attn_outbr = dp("attn_outbr", [1, D], F32, isOutput=False)
    w1aT = dp("w1aT", [D, D], F32, isOutput=False)
    w1bT = dp("w1bT", [D, D], F32, isOutput=False)
    rc_b1r = dp("rc_b1r", [1, D], F32, isOutput=False)
    rc_w2T = dp("rc_w2T", [D, 384], F32, isOutput=False)
    rc_b2 = dp("rc_b2", [384, 1], F32, isOutput=False)
    rc_w3T = dp("rc_w3T", [384, 8], F32, isOutput=False)
    rc_b3r = dp("rc_b3r", [1, 8], F32, isOutput=False)
    # outputs
    oc_out = dp("oc_out", [N, 8], F32, isOutput=True)
    ip_out = dp("ip_out", [N, 1], F32, isOutput=True)
    att_out = dp("att_out", [N, D], F32, isOutput=True)
    attw_out = dp("attw_out", [N, N], F32, isOutput=True)
    rel_out = dp("rel_out", [SLOTS, 8], F32, isOutput=True)

    import contextlib

    with tile.TileContext(nc) as tc, contextlib.ExitStack() as ctx:
        pid = nc.partition_id()
        qv = pid % 4

        const = ctx.enter_context(tc.tile_pool(name="const", bufs=1))
        psA = tc.tile_pool(name="psA", bufs=1, space="PSUM")
        ctxA = contextlib.ExitStack()
        psA = ctxA.enter_context(psA)

        ones_row = const.tile([1, 512], F32, tag="ones_row")
        nc.vector.memset(ones_row[:], 1.0)
        ones_col = const.tile([128, 1], F32, tag="ones_col")
        nc.vector.memset(ones_col[:], 1.0)
        ident = const.tile([128, 128], F32, tag="ident")
        make_identity(nc, ident[:])

        X = []
        for k in range(KT):
            t = const.tile([128, N], F32, tag=f"x{k}")
            nc.sync.dma_start(t[:], xT[128 * k:128 * (k + 1), :])
            X.append(t)
        STt = []
        for s in range(2):
            t = const.tile([128, N], F32, tag=f"st{s}")
            nc.sync.dma_start(t[:], ST[128 * s:128 * (s + 1), :])
            STt.append(t)

        def col_tiles(dram, F, tag):
            """Load [F,1] dram vector as list of [128,1] (or partial) tiles."""
            out = []
            for k in range((F + 127) // 128):
                p = min(128, F - 128 * k)
                t = const.tile([p, 1], F32, tag=f"{tag}{k}")
                nc.sync.dma_start(t[:], dram[128 * k:128 * k + p, :])
                out.append(t)
            return out

        def row_tile(dram, F, tag):
            t = const.tile([1, F], F32, tag=tag)
            nc.sync.dma_start(t[:], dram[:, :])
            return t

        ocb1 = col_tiles(oc_b1, D, "ocb1")
        ocb2 = col_tiles(oc_b2, 384, "ocb2")
        ocb3 = row_tile(oc_b3r, 8, "ocb3")
        ipb1 = col_tiles(ip_b1, 384, "ipb1")
        ipb2 = col_tiles(ip_b2, 192, "ipb2")
        ipb3 = row_tile(ip_b3r, 1, "ipb3")
        bsn = [col_tiles(gnn_bsn[l], D, f"bsn{l}") for l in range(3)]
        gam = [col_tiles(gnn_g[l], D, f"gam{l}") for l in range(3)]
        bet = [col_tiles(gnn_bt[l], D, f"bet{l}") for l in range(3)]
        binr = row_tile(attn_inbr, 3 * D, "binr")
        bor = row_tile(attn_outbr, D, "bor")
        b1r = row_tile(rc_b1r, D, "b1r")
        rcb2 = col_tiles(rc_b2, 384, "rcb2")
        rcb3 = row_tile(rc_b3r, 8, "rcb3")

        act = ctx.enter_context(tc.tile_pool(name="act", bufs=1))

        def load_w(pool, dram, rows, cols, tag):
            out = []
            for k in range((rows + 127) // 128):
                p = min(128, rows - 128 * k)
                t = pool.tile([p, cols], F32, tag=f"{tag}{k}")
                nc.sync.dma_start(t[:], dram[128 * k:128 * k + p, :])
                out.append(t)
            return out

        def mlp_layer(pool_ps, W, X_tiles, n_out, bias_cols, out_tag, n_free=N,
                      act_fn=AF.Relu):
            """yT[m] = act(W.T @ x + b) feature-major; returns sbuf tiles."""
            outs = []
            nk = len(X_tiles)
            for m in range((n_out + 127) // 128):
                mw = min(128, n_out - 128 * m)
                pm = pool_ps.tile([mw, n_free], F32, tag="pmm")
                for k in range(nk):
                    nc.tensor.matmul(pm[:], R(W[k][:, 128 * m:128 * m + mw]),
                                     R(X_tiles[k][:]), start=(k == 0), stop=(k == nk - 1))
                o = act.tile([mw, n_free], F32, tag=f"{out_tag}{m}")
                nc.scalar.activation(o[:], pm[:], act_fn, bias=bias_cols[m][:])
                outs.append(o)
            return outs

        # ---------------- object classifier ----------------
        wA = contextlib.ExitStack()
        wAp = wA.enter_context(tc.tile_pool(name="wA", bufs=1))
        W1 = load_w(wAp, oc_w1T, D, D, "ocw1")
        W2 = load_w(wAp, oc_w2T, D, 384, "ocw2")
        W3 = load_w(wAp, oc_w3T, 384, 8, "ocw3")
        I1 = load_w(wAp, ip_w1T, D, 384, "ipw1")
        I2 = load_w(wAp, ip_w2T, 384, 192, "ipw2")
        I3 = load_w(wAp, ip_w3T, 192, 1, "ipw3")

        H1 = mlp_layer(psA, W1, X, D, ocb1, "och1")
        H2 = mlp_layer(psA, W2, H1, 384, ocb2, "och2")
        for nt in range(2):
            pl = psA.tile([128, 8], F32, tag="phead")
            for k in range(3):
                nc.tensor.matmul(pl[:], R(H2[k][:, 128 * nt:128 * (nt + 1)]),
                                 R(W3[k][:]), start=(k == 0), stop=False)
            nc.tensor.matmul(pl[:], R(ones_row[0:1, 0:128]), R(ocb3[:]),
                             start=False, stop=True)
            ex = act.tile([128, 8], F32, tag="oc_ex")
            sm = act.tile([128, 1], F32, tag="oc_sm")
            nc.scalar.activation(ex[:], pl[:], AF.Exp, accum_out=sm[:])
            rc = act.tile([128, 1], F32, tag="oc_rc")
            nc.vector.reciprocal(rc[:], sm[:])
            pr = act.tile([128, 8], F32, tag="oc_pr")
            nc.vector.tensor_scalar_mul(pr[:], ex[:], rc[:])
            nc.sync.dma_start(oc_out[128 * nt:128 * (nt + 1), :], pr[:])

        # ---------------- importance predictor ----------------
        J1 = mlp_layer(psA, I1, X, 384, ipb1, "iph1")
        J2 = mlp_layer(psA, I2, J1, 192, ipb2, "iph2")
        for nt in range(2):
            pl = psA.tile([128, 1], F32, tag="phead")
            nc.tensor.matmul(pl[:], R(J2[0][:, 128 * nt:128 * (nt + 1)]),
                             R(I3[0][:]), start=True, stop=False)
            nc.tensor.matmul(pl[:], R(J2[1][:, 128 * nt:128 * (nt + 1)]),
                             R(I3[1][:]), start=False, stop=False)
            nc.tensor.matmul(pl[:], R(ones_row[0:1, 0:128]), R(ipb3[:]),
                             start=False, stop=True)
            sg = act.tile([128, 1], F32, tag="ip_sg")
            nc.scalar.activation(sg[:], pl[:], AF.Sigmoid)
            nc.sync.dma_start(ip_out[128 * nt:128 * (nt + 1), :], sg[:])
        wA.close()

        # ---------------- 3 GNN layers ----------------
        wG = contextlib.ExitStack()
        wGp = wG.enter_context(tc.tile_pool(name="wG", bufs=2))
        G = X
        for l in range(3):
            GW = []
            for k in range(KT):
                t = wGp.tile([128, 2 * D], F32, tag=f"gw{k}")
                nc.sync.dma_start(t[:], gnn_w[l][128 * k:128 * (k + 1), :])
                GW.append(t)
            # mT = WnT.T @ g  -> sbuf, then transpose to m_rm
            MRM = [act.tile([128, D], F32, tag=f"mrm{h}") for h in range(2)]
            for m in range(KT):
                pm = psA.tile([128, N], F32, tag="pmm")
                for k in range(KT):
                    nc.tensor.matmul(pm[:], R(GW[k][:, D + 128 * m:D + 128 * (m + 1)]),
                                     R(G[k][:]), start=(k == 0), stop=(k == KT - 1))
                mt = act.tile([128, N], F32, tag=f"mt{m}")
                nc.vector.tensor_copy(mt[:], pm[:])
                for h in range(2):
                    pt = psA.tile([128, 128], F32, tag="paux")
                    nc.tensor.transpose(pt[:], mt[:, 128 * h:128 * (h + 1)], ident[:])
                    nc.vector.tensor_copy(MRM[h][:, 128 * m:128 * (m + 1)], pt[:])
            # o = relu(selfT + neighT + bsn); stats; layernorm
            OT, SQ = [], []
            for m in range(KT):
                po = psA.tile([128, N], F32, tag="pmm")
                for k in range(KT):
                    nc.tensor.matmul(po[:], R(GW[k][:, 128 * m:128 * (m + 1)]),
                                     R(G[k][:]), start=(k == 0), stop=False)
                for s in range(2):
                    nc.tensor.matmul(po[:], R(MRM[s][:, 128 * m:128 * (m + 1)]),
                                     R(STt[s][:]), start=False, stop=(s == 1))
                o = act.tile([128, N], F32, tag=f"ot{m}")
                nc.scalar.activation(o[:], po[:], AF.Relu, bias=bsn[l][m][:])
                OT.append(o)
                sq = act.tile([128, N], F32, tag=f"sq{m}")
                nc.vector.tensor_mul(sq[:], o[:], o[:])
                SQ.append(sq)
            s1 = psA.tile([1, N], F32, tag="pstat")
            s2 = psA.tile([1, N], F32, tag="pstat")
            for k in range(KT):
                nc.tensor.matmul(s1[:], R(ones_col[:]), R(OT[k][:]),
                                 start=(k == 0), stop=(k == KT - 1))
            for k in range(KT):
                nc.tensor.matmul(s2[:], R(ones_col[:]), R(SQ[k][:]),
                                 start=(k == 0), stop=(k == KT - 1))
            mu = act.tile([1, N], F32, tag="mu")
            nc.vector.tensor_scalar_mul(mu[:], s1[:], 1.0 / D)
            msq = act.tile([1, N], F32, tag="msq")
            nc.vector.tensor_scalar_mul(msq[:], s2[:], 1.0 / D)
            mm2 = act.tile([1, N], F32, tag="mm2")
            nc.vector.tensor_mul(mm2[:], mu[:], mu[:])
            var = act.tile([1, N], F32, tag="var")
            nc.vector.tensor_sub(var[:], msq[:], mm2[:])
            sd = act.tile([1, N], F32, tag="sd")
            nc.scalar.activation(sd[:], var[:], AF.Sqrt, bias=EPS)
            rs = act.tile([1, N], F32, tag="rs")
            nc.vector.reciprocal(rs[:], sd[:])
            vv = act.tile([1, N], F32, tag="vv")
            nc.vector.tensor_mul(vv[:], mu[:], rs[:])
            bu = psA.tile([128, N], F32, tag="paux")
            nc.tensor.matmul(bu[:], R(ones_row[0:1, 0:128]), R(rs[:]),
                             start=True, stop=True)
            bv = psA.tile([128, N], F32, tag="paux")
            nc.tensor.matmul(bv[:], R(ones_row[0:1, 0:128]), R(vv[:]),
                             start=True, stop=True)
            Gn = []
            for m in range(KT):
                t1 = act.tile([128, N], F32, tag=f"t1_{m}")
                nc.vector.tensor_mul(t1[:], OT[m][:], bu[:])
                t2 = act.tile([128, N], F32, tag=f"t2_{m}")
                nc.vector.tensor_sub(t2[:], t1[:], bv[:])
                gn = act.tile([128, N], F32, tag=f"g{m}", bufs=2)
                nc.vector.tensor_scalar(gn[:], t2[:], gam[l][m][:], bet[l][m][:],
                                        OP.mult, OP.add)
                Gn.append(gn)
            G = Gn
        wG.close()

        # ---------------- attention ----------------
        wAt = contextlib.ExitStack()
        wAtp = wAt.enter_context(tc.tile_pool(name="wAt", bufs=1))
        WIN = load_w(wAtp, attn_inT, D, 3 * D, "win")
        WOUT = []
        for h in range(HEADS):
            t = wAtp.tile([HD, D], F32, tag=f"wout{h}")
            nc.sync.dma_start(t[:], attn_outT[HD * h:HD * (h + 1), :])
            WOUT.append(t)

        QK = []
        for mh in range(16):
            c0 = HD * mh if mh < 8 else D + HD * (mh - 8)
            pq = psA.tile([HD, N], F32, tag="pmm")
            for k in range(KT):
                nc.tensor.matmul(pq[:], R(WIN[k][:, c0:c0 + HD]), R(G[k][:]),
                                 start=(k == 0), stop=False)
            nc.tensor.matmul(pq[:], R(binr[0:1, c0:c0 + HD]), R(ones_row[0:1, 0:N]),
                             start=False, stop=True)
            t = act.tile([HD, N], F32, tag=f"qk{mh}")
            nc.vector.tensor_copy(t[:], pq[:])
            QK.append(t)
        VR = []
        for nt in range(2):
            t = act.tile([128, D], F32, tag=f"vr{nt}")
            for nch in range(2):
                c0 = 2 * D + 384 * nch
                pv = psA.tile([128, 384], F32, tag="pmm")
                for k in range(KT):
                    nc.tensor.matmul(pv[:], R(G[k][:, 128 * nt:128 * (nt + 1)]),
                                     R(WIN[k][:, c0:c0 + 384]),
                                     start=(k == 0), stop=False)
                nc.tensor.matmul(pv[:], R(ones_row[0:1, 0:128]),
                                 R(binr[0:1, c0:c0 + 384]), start=False, stop=True)
                nc.vector.tensor_copy(t[:, 384 * nch:384 * (nch + 1)], pv[:])
            VR.append(t)

        ACC = [act.tile([128, N], F32, tag=f"acc{kb}") for kb in range(2)]
        CT = []
        for h in range(HEADS):
            AT = []
            ssum = psA.tile([1, N], F32, tag="pstat")
            EXP = []
            for kb in range(2):
                ps_ = psA.tile([128, N], F32, tag="pmm")
                nc.tensor.matmul(ps_[:], R(QK[8 + h][:, 128 * kb:128 * (kb + 1)]),
                                 R(QK[h][:]), start=True, stop=True)
                e = act.tile([128, N], F32, tag=f"e{kb}")
                nc.scalar.activation(e[:], ps_[:], AF.Exp, scale=1.0 / float(np.sqrt(HD)))
                EXP.append(e)
                nc.tensor.matmul(ssum[:], R(ones_col[:]), R(e[:]),
                                 start=(kb == 0), stop=(kb == 1))
            rsum = act.tile([1, N], F32, tag="rsum")
            nc.vector.reciprocal(rsum[:], ssum[:])
            br = psA.tile([128, N], F32, tag="paux")
            nc.tensor.matmul(br[:], R(ones_row[0:1, 0:128]), R(rsum[:]),
                             start=True, stop=True)
            pc = psA.tile([HD, N], F32, tag="pctx")
            for kb in range(2):
                a = act.tile([128, N], F32, tag=f"a{kb}")
                nc.vector.tensor_mul(a[:], EXP[kb][:], br[:])
                AT.append(a)
                if h == 0:
                    nc.vector.tensor_copy(ACC[kb][:], a[:])
                else:
                    nc.vector.tensor_add(ACC[kb][:], ACC[kb][:], a[:])
                nc.tensor.matmul(pc[:], R(VR[kb][:, HD * h:HD * (h + 1)]), R(a[:]),
                                 start=(kb == 0), stop=(kb == 1))
            ct = act.tile([HD, N], F32, tag=f"ct{h}")
            nc.vector.tensor_copy(ct[:], pc[:])
            CT.append(ct)

        ATT = []
        for m in range(KT):
            pa = psA.tile([128, N], F32, tag="pmm")
            for h in range(HEADS):
                nc.tensor.matmul(pa[:], R(WOUT[h][:, 128 * m:128 * (m + 1)]),
                                 R(CT[h][:]), start=(h == 0), stop=False)
            nc.tensor.matmul(pa[:], R(bor[0:1, 128 * m:128 * (m + 1)]),
                             R(ones_row[0:1, 0:N]), start=False, stop=True)
            at = act.tile([128, N], F32, tag=f"att{m}")
            nc.vector.tensor_copy(at[:], pa[:])
            ATT.append(at)
        # attended row-major out
        for nt in range(2):
            ar = act.tile([128, D], F32, tag=f"ar{nt}")
            for m in range(KT):
                pt = psA.tile([128, 128], F32, tag="paux")
                nc.tensor.transpose(pt[:], ATT[m][:, 128 * nt:128 * (nt + 1)], ident[:])
                nc.vector.tensor_copy(ar[:, 128 * m:128 * (m + 1)], pt[:])
            nc.sync.dma_start(att_out[128 * nt:128 * (nt + 1), :], ar[:])
        # attn weights (mean over heads, transposed back to [q, k])
        for qt in range(2):
            aw = act.tile([128, N], F32, tag=f"aw{qt}")
            for kb in range(2):
                pt = psA.tile([128, 128], F32, tag="paux")
                nc.tensor.transpose(pt[:], ACC[kb][:, 128 * qt:128 * (qt + 1)], ident[:])
                nc.vector.tensor_scalar_mul(aw[:, 128 * kb:128 * (kb + 1)], pt[:],
                                            1.0 / HEADS)
            nc.sync.dma_start(attw_out[128 * qt:128 * (qt + 1), :], aw[:])
        wAt.close()

        # ---------------- A1b / A2 ----------------
        wR = contextlib.ExitStack()
        wRp = wR.enter_context(tc.tile_pool(name="wR", bufs=1))
        WA = load_w(wRp, w1aT, D, D, "w1a")
        WB = load_w(wRp, w1bT, D, D, "w1b")
        A1B, A2 = [], []
        for m in range(KT):
            p1 = psA.tile([128, N], F32, tag="pmm")
            for k in range(KT):
                nc.tensor.matmul(p1[:], R(WA[k][:, 128 * m:128 * (m + 1)]),
                                 R(ATT[k][:]), start=(k == 0), stop=False)
            nc.tensor.matmul(p1[:], R(b1r[0:1, 128 * m:128 * (m + 1)]),
                             R(ones_row[0:1, 0:N]), start=False, stop=True)
            a1 = act.tile([128, N], F32, tag=f"a1b{m}")
            nc.vector.tensor_copy(a1[:], p1[:])
            A1B.append(a1)
            p2 = psA.tile([128, N], F32, tag="pmm")
            for k in range(KT):
                nc.tensor.matmul(p2[:], R(WB[k][:, 128 * m:128 * (m + 1)]),
                                 R(ATT[k][:]), start=(k == 0), stop=(k == KT - 1))
            a2 = act.tile([128, N], F32, tag=f"a2_{m}")
            nc.vector.tensor_copy(a2[:], p2[:])
            A2.append(a2)
        RW2 = load_w(wRp, rc_w2T, D, 384, "rw2")
        RW3 = load_w(wRp, rc_w3T, 384, 8, "rw3")
        ctxA.close()  # free phase-A psum

        # ---------------- phase B: relation MLP over pair tiles ----------------
        psB = ctx.enter_context(tc.tile_pool(name="psB", bufs=1, space="PSUM"))
        pb_pool = ctx.enter_context(tc.tile_pool(name="pb", bufs=1))

        for q in range(4):
            tiles = _segments(q)
            with tc.If(qv == q):
                for t in range(NTILES):
                    segs = tiles[t]
                    h1 = [pb_pool.tile([128, 512], F32, tag=f"h1_{k}", bufs=2)
                          for k in range(KT)]
                    for k in range(KT):
                        for (ic, j0, ln, dst) in segs:
                            if k < 3:
                                nc.scalar.activation(
                                    h1[k][:, dst:dst + ln],
                                    A2[k][:, j0:j0 + ln], AF.Relu,
                                    bias=A1B[k][:, ic:ic + 1])
                            else:
                                nc.vector.tensor_scalar(
                                    h1[k][:, dst:dst + ln],
                                    A2[k][:, j0:j0 + ln],
                                    A1B[k][:, ic:ic + 1], 0.0,
                                    OP.add, OP.max)
                    h2 = []
                    for m in range(3):
                        pm = psB.tile([128, 512], F32, tag="ph2", bufs=3)
                        for k in range(KT):
                            nc.tensor.matmul(pm[:], R(RW2[k][:, 128 * m:128 * (m + 1)]),
                                             R(h1[k][:]), start=(k == 0),
                                             stop=(k == KT - 1))
                        o = pb_pool.tile([128, 512], F32, tag=f"h2_{m}", bufs=2)
                        nc.scalar.activation(o[:], pm[:], AF.Relu, bias=rcb2[m][:])
                        h2.append(o)
                    pl = psB.tile([128, 32], F32, tag="plog", bufs=2)
                    for pb in range(4):
                        for k in range(3):
                            nc.tensor.matmul(pl[:, 8 * pb:8 * (pb + 1)],
                                             R(h2[k][:, 128 * pb:128 * (pb + 1)]),
                                             R(RW3[k][:]), start=(k == 0), stop=False)
                        nc.tensor.matmul(pl[:, 8 * pb:8 * (pb + 1)],
                                         R(ones_row[0:1, 0:128]), R(rcb3[:]),
                                         start=False, stop=True)
                    ex = pb_pool.tile([128, 32], F32, tag="ex", bufs=2)
                    nc.scalar.activation(ex[:], pl[:], AF.Exp)
                    sm = pb_pool.tile([128, 4], F32, tag="sm", bufs=2)
                    nc.vector.tensor_reduce(
                        out=sm[:], in_=ex[:].rearrange("p (g x) -> p g x", g=4),
                        axis=mybir.AxisListType.X, op=OP.add)
                    rcp = pb_pool.tile([128, 4], F32, tag="rcp", bufs=2)
                    nc.vector.reciprocal(rcp[:], sm[:])
                    pr = pb_pool.tile([128, 32], F32, tag="pr", bufs=2)
                    for g in range(4):
                        nc.vector.tensor_scalar_mul(pr[:, 8 * g:8 * (g + 1)],
                                                    ex[:, 8 * g:8 * (g + 1)],
                                                    rcp[:, g:g + 1])
                    nc.sync.dma_start(
                        rel_out[512 * t:512 * (t + 1), :].rearrange(
                            "(pb p) c -> p pb c", p=128),
                        pr[:].rearrange("p (pb c) -> p pb c", pb=4))
    return nc


def get_program():
    if "nc" not in _prog_cache:
        _prog_cache["nc"] = _build_program()
    return _prog_cache["nc"]


def make_in_maps(inputs):
    f = lambda a: np.ascontiguousarray(np.asarray(a), dtype=np.float32)
    x = f(inputs["node_embeddings"])
    ew = f(inputs["edge_weights"])
    ei = np.asarray(inputs["edge_indices"])
    src, tgt = ei[:, 0], ei[:, 1]
    ST = np.zeros((N, N), np.float32)
    np.add.at(ST, (src, tgt), ew)

    com = {}
    com["ST"] = ST
    com["oc_w1T"] = f(inputs["oc_w1"].T)
    com["oc_b1"] = f(inputs["oc_b1"]).reshape(D, 1)
    com["oc_w2T"] = f(inputs["oc_w2"].T)
    com["oc_b2"] = f(inputs["oc_b2"]).reshape(384, 1)
    com["oc_w3T"] = f(inputs["oc_w3"].T)
    com["oc_b3r"] = f(inputs["oc_b3"]).reshape(1, 8)
    com["ip_w1T"] = f(inputs["ip_w1"].T)
    com["ip_b1"] = f(inputs["ip_b1"]).reshape(384, 1)
    com["ip_w2T"] = f(inputs["ip_w2"].T)
    com["ip_b2"] = f(inputs["ip_b2"]).reshape(192, 1)
    com["ip_w3T"] = f(inputs["ip_w3"].T)
    com["ip_b3r"] = f(inputs["ip_b3"]).reshape(1, 1)
    for l in range(3):
        com[f"gnn{l}_w"] = np.concatenate(
            [f(inputs[f"gnn{l}_ws"].T), f(inputs[f"gnn{l}_wn"].T)], axis=1)
        com[f"gnn{l}_bsn"] = (f(inputs[f"gnn{l}_bs"]) +
                              f(inputs[f"gnn{l}_bn"])).reshape(D, 1)
        com[f"gnn{l}_gm"] = f(inputs[f"gnn{l}_g"]).reshape(D, 1)
        com[f"gnn{l}_bt"] = f(inputs[f"gnn{l}_beta"]).reshape(D, 1)
    com["attn_inT"] = f(inputs["attn_in_w"].T)
    com["attn_inbr"] = f(inputs["attn_in_b"]).reshape(1, 3 * D)
    com["attn_outT"] = f(inputs["attn_out_w"].T)
    com["attn_outbr"] = f(inputs["attn_out_b"]).reshape(1, D)
    rc_w1 = f(inputs["rc_w1"])
    com["w1aT"] = np.ascontiguousarray(rc_w1[:, :D].T)
    com["w1bT"] = np.ascontiguousarray(rc_w1[:, D:].T)
    com["rc_b1r"] = f(inputs["rc_b1"]).reshape(1, D)
    com["rc_w2T"] = f(inputs["rc_w2"].T)
    com["rc_b2"] = f(inputs["rc_b2"]).reshape(384, 1)
    com["rc_w3T"] = f(inputs["rc_w3"].T)
    com["rc_b3r"] = f(inputs["rc_b3"]).reshape(1, 8)

    xT = [np.ascontiguousarray(x[b].T) for b in range(B)]
    in_maps = []
    for c in range(NCORES):
        m = dict(com)
        m["xT"] = xT[c // 4]
        in_maps.append(m)
    return in_maps


def assemble(results):
    obj = np.stack([results[0]["oc_out"], results[4]["oc_out"]])
    imp = np.stack([results[0]["ip_out"], results[4]["ip_out"]])
    att = np.stack([results[0]["att_out"], results[4]["att_out"]])
    attw = np.stack([results[0]["attw_out"], results[4]["attw_out"]])
    rel = np.zeros((B, P, 8), np.float32)
    base = lambda i: i * 255 - i * (i - 1) // 2
    for c in range(NCORES):
        b, q = c // 4, c % 4
        ro = results[c]["rel_out"]
        for k, u in enumerate(_units(q)):
            s0 = 256 * k
            rel[b, base(u):base(u) + 255 - u] = ro[s0:s0 + 255 - u]
            rel[b, base(254 - u):base(254 - u) + u + 1] = ro[s0 + 255 - u:s0 + 256]
    return obj, rel, imp, att, attw


def kernel(**inputs):
    from concourse.bass_utils import run_bass_kernel_spmd
    nc = get_program()
    in_maps = make_in_maps(inputs)
    res = run_bass_kernel_spmd(nc, in_maps, list(range(NCORES))).results
    return assemble(res)


# revision 47
# speedup vs baseline: 1.0678x; 1.0678x over previous
"""Trainium2 Bass kernel for AdvancedNeuralStructureAnalyzer.

Sharding: 8 cores = 2 batches x 4 pair-quarters.
Core c: batch b=c//4 computes full phase A (heads, 3 GNN layers, attention)
redundantly within its batch group; pair-quarter q=c%4 computes the relation
MLP for units u = q+4k (mirror-paired i-blocks of uniform 256-pair size).

Device layout: activations feature-major ("T" = [features->partitions,
nodes->free]). All weights host-pre-transposed to [in_features, out_features]
so no on-device weight transposes are needed. Matmul operands are float32r
(full-rate PE); their producers write f32r-typed tiles so the BIR verifier's
rounded-producer rule is satisfied.
"""
import contextlib

import numpy as np

B, N, D, E, HEADS = 2, 256, 768, 8192, 8
HD = D // HEADS            # 96
KT = D // 128              # 6
EPS = 1e-5
P = N * (N - 1) // 2       # 32640
NCORES = 8
SLOTS = 8192               # per-core padded pair slots (32 units x 256)
NTILES = 16                # 512-slot tiles per core

_prog_cache = {}


def _units(q):
    return [q + 4 * k for k in range(32)]


def _segments(q):
    """Per 512-tile: list of (i_col, j0, ln, dst). Unit u at base beta:
    (u, u+1, 255-u, beta) and (254-u, 255-u, u+1, beta+255-u)."""
    us = _units(q)
    tiles = []
    for t in range(NTILES):
        segs = []
        for s, u in enumerate((us[2 * t], us[2 * t + 1])):
            beta = 256 * s
            segs.append((u, u + 1, 255 - u, beta))
            segs.append((254 - u, 255 - u, u + 1, beta + 255 - u))
        tiles.append(segs)
    return tiles


def _build_program(fixed_q=None):
    import concourse.tile as tile
    from concourse import bacc, mybir
    from concourse.masks import make_identity

    F32 = mybir.dt.float32
    F32R = mybir.dt.float32r
    AF = mybir.ActivationFunctionType
    OP = mybir.AluOpType

    nc = bacc.Bacc("TRN2", target_bir_lowering=False, debug=False,
                   num_devices=NCORES)
    dp = nc.declare_dram_parameter
    xT = dp("xT", [D, N], F32R, isOutput=False)
    ST = dp("ST", [N, N], F32R, isOutput=False)
    ones_in = dp("ones_in", [1, 512], F32R, isOutput=False)
    onesc_in = dp("onesc_in", [128, 1], F32R, isOutput=False)
    # packed per-partition bias columns [128, NCOLS] and packed rows [1, NROWS]
    bcols_in = dp("bcols_in", [128, 71], F32, isOutput=False)
    brows_in = dp("brows_in", [1, 3857], F32R, isOutput=False)
    oc_w1T = dp("oc_w1T", [D, D], F32R, isOutput=False)
    oc_w2T = dp("oc_w2T", [D, 384], F32R, isOutput=False)
    oc_w3T = dp("oc_w3T", [384, 8], F32R, isOutput=False)
    ip_w1T = dp("ip_w1T", [D, 384], F32R, isOutput=False)
    ip_w2T = dp("ip_w2T", [384, 192], F32R, isOutput=False)
    ip_w3T = dp("ip_w3T", [192, 1], F32R, isOutput=False)
    gnn_w = [dp(f"gnn{l}_w", [D, 2 * D], F32R, isOutput=False) for l in range(3)]
    attn_inT = dp("attn_inT", [D, 3 * D], F32R, isOutput=False)
    attn_outT = dp("attn_outT", [D, D], F32R, isOutput=False)
    w1aT = dp("w1aT", [D, D], F32R, isOutput=False)
    w1bT = dp("w1bT", [D, D], F32R, isOutput=False)
    rc_w2T = dp("rc_w2T", [D, 384], F32R, isOutput=False)
    rc_w3T = dp("rc_w3T", [384, 8], F32R, isOutput=False)
    oc_out = dp("oc_out", [N, 8], F32, isOutput=True)
    ip_out = dp("ip_out", [N, 1], F32, isOutput=True)
    att_out = dp("att_out", [N, D], F32, isOutput=True)
    attw_out = dp("attw_out", [N, N], F32, isOutput=True)
    rel_out = dp("rel_out", [8, SLOTS], F32, isOutput=True)

    with tile.TileContext(nc) as tc, contextlib.ExitStack() as ctx, \
            nc.allow_low_precision(reason="fp32r matmul input rounding"):
        pid = nc.partition_id()
        qv = pid % 4

        const = ctx.enter_context(tc.tile_pool(name="const", bufs=1))
        pRel = ctx.enter_context(tc.tile_pool(name="pRel", bufs=1))
        wRB = ctx.enter_context(tc.tile_pool(name="wRB", bufs=1))
        sA = contextlib.ExitStack()
        psA = sA.enter_context(tc.tile_pool(name="psA", bufs=1, space="PSUM"))

        def pmm(shape):
            return psA.tile(shape, F32, tag="pmm", bufs=3, name="pmm")

        def pstat(shape):
            return psA.tile(shape, F32, tag="pstat", bufs=2, name="pstat")

        def paux(shape):
            return psA.tile(shape, F32, tag="paux", bufs=2, name="paux")

        ones_row = const.tile([1, 512], F32R, tag="ones_row", name="ones_row")
        nc.sync.dma_start(ones_row[:], ones_in[:, :])
        ones_col = const.tile([128, 1], F32R, tag="ones_col", name="ones_col")
        nc.sync.dma_start(ones_col[:], onesc_in[:, :])
        ident = const.tile([128, 128], F32, tag="ident", name="ident")
        make_identity(nc, ident[:])
        epst = const.tile([1, 1], F32, tag="epst", name="epst")
        nc.vector.memset(epst[:], EPS)

        X = []
        for k in range(KT):
            t = const.tile([128, N], F32R, tag=f"x{k}", name=f"x{k}")
            nc.sync.dma_start(t[:], xT[128 * k:128 * (k + 1), :])
            X.append(t)
        STt = []
        for s in range(2):
            t = const.tile([128, N], F32R, tag=f"st{s}", name=f"st{s}")
            nc.sync.dma_start(t[:], ST[128 * s:128 * (s + 1), :])
            STt.append(t)

        BC = const.tile([128, 71], F32, tag="bcols", name="bcols")
        nc.sync.dma_start(BC[:], bcols_in[:, :])
        BR = const.tile([1, 3857], F32R, tag="brows", name="brows")
        nc.sync.dma_start(BR[:], brows_in[:, :])

        def cols(c0, F):
            return [BC[0:min(128, F - 128 * k), c0 + k:c0 + k + 1]
                    for k in range((F + 127) // 128)]

        ocb1 = cols(0, D)
        ocb2 = cols(6, 384)
        ipb1 = cols(9, 384)
        ipb2 = cols(12, 192)
        bsn = [cols(14 + 18 * l, D) for l in range(3)]
        gam = [cols(20 + 18 * l, D) for l in range(3)]
        bet = [cols(26 + 18 * l, D) for l in range(3)]
        rcb2 = cols(68, 384)
        ocb3 = BR[0:1, 0:8]
        ipb3 = BR[0:1, 8:9]
        binr = BR[0:1, 9:2313]
        bor = BR[0:1, 2313:3081]
        b1r = BR[0:1, 3081:3849]
        rcb3 = BR[0:1, 3849:3857]

        def load_w(pool, dram, rows, cols, tag):
            out = []
            for k in range((rows + 127) // 128):
                p = min(128, rows - 128 * k)
                t = pool.tile([p, cols], F32R, tag=f"{tag}{k}", name=f"{tag}{k}")
                nc.sync.dma_start(t[:], dram[128 * k:128 * k + p, :])
                out.append(t)
            return out

        pG = contextlib.ExitStack()
        pGp = pG.enter_context(tc.tile_pool(name="pG", bufs=1))

        # ---------------- object classifier + importance ----------------
        sHeads = contextlib.ExitStack()
        wAp = sHeads.enter_context(tc.tile_pool(name="wA", bufs=1))
        pH = sHeads.enter_context(tc.tile_pool(name="pH", bufs=1))
        W1 = load_w(wAp, oc_w1T, D, D, "ocw1")
        W2 = load_w(wAp, oc_w2T, D, 384, "ocw2")
        W3 = load_w(wAp, oc_w3T, 384, 8, "ocw3")
        I1 = load_w(wAp, ip_w1T, D, 384, "ipw1")
        I2 = load_w(wAp, ip_w2T, 384, 192, "ipw2")
        I3 = load_w(wAp, ip_w3T, 192, 1, "ipw3")

        def mlp_layer(W, X_tiles, n_out, bias_cols, out_tag):
            outs = []
            nk = len(X_tiles)
            for m in range((n_out + 127) // 128):
                mw = min(128, n_out - 128 * m)
                pm = pmm([mw, N])
                for k in range(nk):
                    nc.tensor.matmul(pm[:], W[k][:, 128 * m:128 * m + mw],
                                     X_tiles[k][:], start=(k == 0),
                                     stop=(k == nk - 1))
                o = pH.tile([mw, N], F32R, tag=f"{out_tag}{m}", name=f"{out_tag}{m}")
                nc.scalar.activation(o[:], pm[:], AF.Relu, bias=bias_cols[m])
                outs.append(o)
            return outs

        H1 = mlp_layer(W1, X, D, ocb1, "och1")
        H2 = mlp_layer(W2, H1, 384, ocb2, "och2")
        for nt in range(2):
            pl = pstat([128, 8])
            for k in range(3):
                nc.tensor.matmul(pl[:], H2[k][:, 128 * nt:128 * (nt + 1)],
                                 W3[k][:], start=(k == 0), stop=False)
            nc.tensor.matmul(pl[:], ones_row[0:1, 0:128], ocb3,
                             start=False, stop=True)
            ex = pH.tile([128, 8], F32, tag="oc_ex", bufs=2, name="oc_ex")
            sm = pH.tile([128, 1], F32, tag="oc_sm", bufs=2, name="oc_sm")
            nc.scalar.activation(ex[:], pl[:], AF.Exp, accum_out=sm[:])
            rc = pH.tile([128, 1], F32, tag="oc_rc", bufs=2, name="oc_rc")
            nc.vector.reciprocal(rc[:], sm[:])
            pr = pH.tile([128, 8], F32, tag="oc_pr", bufs=2, name="oc_pr")
            nc.vector.tensor_scalar_mul(pr[:], ex[:], rc[:])
            nc.sync.dma_start(oc_out[128 * nt:128 * (nt + 1), :], pr[:])

        J1 = mlp_layer(I1, X, 384, ipb1, "iph1")
        J2 = mlp_layer(I2, J1, 192, ipb2, "iph2")
        for nt in range(2):
            pl = pstat([128, 1])
            nc.tensor.matmul(pl[:], J2[0][:, 128 * nt:128 * (nt + 1)].bitcast(F32),
                             I3[0][:].bitcast(F32), start=True, stop=False)
            nc.tensor.matmul(pl[:], J2[1][:, 128 * nt:128 * (nt + 1)].bitcast(F32),
                             I3[1][:].bitcast(F32), start=False, stop=False)
            nc.tensor.matmul(pl[:], ones_row[0:1, 0:128].bitcast(F32),
                             ipb3.bitcast(F32), start=False, stop=True)
            sg = pH.tile([128, 1], F32, tag="ip_sg", bufs=2, name="ip_sg")
            nc.scalar.activation(sg[:], pl[:], AF.Sigmoid)
            nc.sync.dma_start(ip_out[128 * nt:128 * (nt + 1), :], sg[:])
        sHeads.close()

        # ---------------- 3 GNN layers ----------------
        sG = contextlib.ExitStack()
        wGp = sG.enter_context(tc.tile_pool(name="wG", bufs=1))
        pGnn = sG.enter_context(tc.tile_pool(name="pGnn", bufs=1))
        G = X
        for l in range(3):
            GW = []
            for k in range(KT):
                t = wGp.tile([128, 2 * D], F32R, tag=f"gw{k}", bufs=2,
                             name=f"gw{k}")
                nc.sync.dma_start(t[:], gnn_w[l][128 * k:128 * (k + 1), :])
                GW.append(t)
            MRM = [pGnn.tile([128, D], F32R, tag=f"mrm{h}", name=f"mrm{h}")
                   for h in range(2)]
            for m in range(KT):
                pm = pmm([128, N])
                for k in range(KT):
                    nc.tensor.matmul(pm[:], GW[k][:, D + 128 * m:D + 128 * (m + 1)],
                                     G[k][:], start=(k == 0), stop=(k == KT - 1))
                mt = pGnn.tile([128, N], F32, tag=f"mt{m}", name=f"mt{m}")
                nc.scalar.copy(mt[:], pm[:])
                for h in range(2):
                    pt = paux([128, 128])
                    nc.tensor.transpose(pt[:], mt[:, 128 * h:128 * (h + 1)],
                                        ident[:])
                    nc.scalar.copy(MRM[h][:, 128 * m:128 * (m + 1)], pt[:])
            OT, SQ = [], []
            for m in range(KT):
                po = pmm([128, N])
                for k in range(KT):
                    nc.tensor.matmul(po[:], GW[k][:, 128 * m:128 * (m + 1)],
                                     G[k][:], start=(k == 0), stop=False)
                for s in range(2):
                    nc.tensor.matmul(po[:], MRM[s][:, 128 * m:128 * (m + 1)],
                                     STt[s][:], start=False, stop=(s == 1))
                o = pGnn.tile([128, N], F32R, tag=f"ot{m}", name=f"ot{m}")
                nc.scalar.activation(o[:], po[:], AF.Relu, bias=bsn[l][m])
                OT.append(o)
                sq = pGnn.tile([128, N], F32R, tag=f"sq{m}", name=f"sq{m}")
                nc.vector.tensor_mul(sq[:], o[:], o[:])
                SQ.append(sq)
            s1 = pstat([1, N])
            s2 = pstat([1, N])
            for k in range(KT):
                nc.tensor.matmul(s1[:], ones_col[:], OT[k][:],
                                 start=(k == 0), stop=(k == KT - 1))
            for k in range(KT):
                nc.tensor.matmul(s2[:], ones_col[:], SQ[k][:],
                                 start=(k == 0), stop=(k == KT - 1))
            mu = pGnn.tile([1, N], F32, tag="mu", name="mu")
            nc.vector.tensor_scalar_mul(mu[:], s1[:], 1.0 / D)
            msq = pGnn.tile([1, N], F32, tag="msq", name="msq")
            nc.vector.tensor_scalar_mul(msq[:], s2[:], 1.0 / D)
            mm2 = pGnn.tile([1, N], F32, tag="mm2", name="mm2")
            nc.vector.tensor_mul(mm2[:], mu[:], mu[:])
            var = pGnn.tile([1, N], F32, tag="var", name="var")
            nc.vector.tensor_sub(var[:], msq[:], mm2[:])
            sd = pGnn.tile([1, N], F32, tag="sd", name="sd")
            nc.scalar.activation(sd[:], var[:], AF.Sqrt, bias=epst[:])
            rs = pGnn.tile([1, N], F32R, tag="rs", name="rs")
            nc.vector.reciprocal(rs[:], sd[:])
            vv = pGnn.tile([1, N], F32R, tag="vv", name="vv")
            nc.vector.tensor_mul(vv[:], mu[:], rs[:].bitcast(F32))
            bu = paux([128, N])
            nc.tensor.matmul(bu[:], ones_row[0:1, 0:128], rs[:],
                             start=True, stop=True)
            bv = paux([128, N])
            nc.tensor.matmul(bv[:], ones_row[0:1, 0:128], vv[:],
                             start=True, stop=True)
            Gn = []
            for m in range(KT):
                t1 = pGnn.tile([128, N], F32, tag=f"t1_{m}", name=f"t1_{m}")
                nc.vector.tensor_mul(t1[:], OT[m][:].bitcast(F32), bu[:])
                t2 = pGnn.tile([128, N], F32, tag=f"t2_{m}", name=f"t2_{m}")
                nc.vector.tensor_sub(t2[:], t1[:], bv[:])
                gn = pGp.tile([128, N], F32R, tag=f"g{m}", bufs=2, name=f"g{m}")
                nc.vector.tensor_scalar(gn[:], t2[:], gam[l][m], bet[l][m],
                                        OP.mult, OP.add)
                Gn.append(gn)
            G = Gn
        sG.close()

        # ---------------- attention ----------------
        sAt = contextlib.ExitStack()
        pAt = sAt.enter_context(tc.tile_pool(name="pAt", bufs=1))
        sW1 = contextlib.ExitStack()
        wAtp = sW1.enter_context(tc.tile_pool(name="wAt1", bufs=1))
        WIN = load_w(wAtp, attn_inT, D, 3 * D, "win")
        WOUT = []
        for h in range(HEADS):
            t = wAtp.tile([HD, D], F32R, tag=f"wout{h}", name=f"wout{h}")
            nc.sync.dma_start(t[:], attn_outT[HD * h:HD * (h + 1), :])
            WOUT.append(t)
        # relation weights (phase B): load after attention weights
        RW2 = load_w(wRB, rc_w2T, D, 384, "rw2")
        RW3 = load_w(wRB, rc_w3T, 384, 8, "rw3")

        QK = []
        for mh in range(16):
            c0 = HD * mh if mh < 8 else D + HD * (mh - 8)
            pq = pmm([HD, N])
            for k in range(KT):
                nc.tensor.matmul(pq[:], WIN[k][:, c0:c0 + HD], G[k][:],
                                 start=(k == 0), stop=False)
            nc.tensor.matmul(pq[:], binr[0:1, c0:c0 + HD], ones_row[0:1, 0:N],
                             start=False, stop=True)
            t = pAt.tile([HD, N], F32R, tag=f"qk{mh}", name=f"qk{mh}")
            nc.scalar.copy(t[:], pq[:])
            QK.append(t)
        VR = []
        for nt in range(2):
            t = pAt.tile([128, D], F32R, tag=f"vr{nt}", name=f"vr{nt}")
            for nch in range(2):
                c0 = 2 * D + 384 * nch
                pv = pmm([128, 384])
                for k in range(KT):
                    nc.tensor.matmul(pv[:], G[k][:, 128 * nt:128 * (nt + 1)],
                                     WIN[k][:, c0:c0 + 384],
                                     start=(k == 0), stop=False)
                nc.tensor.matmul(pv[:], ones_row[0:1, 0:128],
                                 binr[0:1, c0:c0 + 384], start=False, stop=True)
                nc.scalar.copy(t[:, 384 * nch:384 * (nch + 1)], pv[:])
            VR.append(t)

        ACC = [pAt.tile([128, N], F32, tag=f"acc{kb}", name=f"acc{kb}")
               for kb in range(2)]
        CT = []
        for h in range(HEADS):
            ssum = pstat([1, N])
            EXP = []
            for kb in range(2):
                ps_ = pmm([128, N])
                nc.tensor.matmul(ps_[:], QK[8 + h][:, 128 * kb:128 * (kb + 1)],
                                 QK[h][:], start=True, stop=True)
                e = pAt.tile([128, N], F32R, tag=f"e{kb}", bufs=2, name=f"e{kb}")
                nc.scalar.activation(e[:], ps_[:], AF.Exp,
                                     scale=1.0 / float(np.sqrt(HD)))
                EXP.append(e)
                nc.tensor.matmul(ssum[:], ones_col[:], e[:],
                                 start=(kb == 0), stop=(kb == 1))
            rsum = pAt.tile([1, N], F32R, tag="rsum", bufs=2, name="rsum")
            nc.vector.reciprocal(rsum[:], ssum[:])
            br = paux([128, N])
            nc.tensor.matmul(br[:], ones_row[0:1, 0:128], rsum[:],
                             start=True, stop=True)
            pc = paux([HD, N])
            for kb in range(2):
                a = pAt.tile([128, N], F32R, tag=f"a{kb}", bufs=2, name=f"a{kb}")
                nc.vector.tensor_mul(a[:], EXP[kb][:].bitcast(F32), br[:])
                if h == 0:
                    nc.vector.tensor_copy(ACC[kb][:], a[:].bitcast(F32))
                else:
                    nc.vector.tensor_add(ACC[kb][:], ACC[kb][:],
                                         a[:].bitcast(F32))
                nc.tensor.matmul(pc[:], VR[kb][:, HD * h:HD * (h + 1)], a[:],
                                 start=(kb == 0), stop=(kb == 1))
            ct = pAt.tile([HD, N], F32R, tag=f"ct{h}", name=f"ct{h}")
            nc.scalar.copy(ct[:], pc[:])
            CT.append(ct)

        ATT = []
        for m in range(KT):
            pa = pmm([128, N])
            for h in range(HEADS):
                nc.tensor.matmul(pa[:], WOUT[h][:, 128 * m:128 * (m + 1)],
                                 CT[h][:], start=(h == 0), stop=False)
            nc.tensor.matmul(pa[:], bor[0:1, 128 * m:128 * (m + 1)],
                             ones_row[0:1, 0:N], start=False, stop=True)
            at = pAt.tile([128, N], F32R, tag=f"att{m}", name=f"att{m}")
            nc.scalar.copy(at[:], pa[:])
            ATT.append(at)
        for nt in range(2):
            for m in range(KT):
                pt = paux([128, 128])
                nc.tensor.transpose(
                    pt[:], ATT[m][:, 128 * nt:128 * (nt + 1)].bitcast(F32),
                    ident[:])
                stg = pAt.tile([128, 128], F32, tag="stg", bufs=3, name="stg")
                nc.vector.tensor_copy(stg[:], pt[:])
                nc.sync.dma_start(
                    att_out[128 * nt:128 * (nt + 1), 128 * m:128 * (m + 1)],
                    stg[:])
        for kb in range(2):
            nc.vector.tensor_scalar_mul(ACC[kb][:], ACC[kb][:], 1.0 / HEADS)
        for qt in range(2):
            for kb in range(2):
                pt = paux([128, 128])
                nc.tensor.transpose(pt[:], ACC[kb][:, 128 * qt:128 * (qt + 1)],
                                    ident[:])
                stg = pAt.tile([128, 128], F32, tag="stg", bufs=3, name="stg")
                nc.vector.tensor_copy(stg[:], pt[:])
                nc.sync.dma_start(
                    attw_out[128 * qt:128 * (qt + 1), 128 * kb:128 * (kb + 1)],
                    stg[:])
        sW1.close()

        # ---------------- A1b / A2 ----------------
        sW2 = contextlib.ExitStack()
        wRp = sW2.enter_context(tc.tile_pool(name="wAt2", bufs=1))
        WA = load_w(wRp, w1aT, D, D, "w1a")
        WB = load_w(wRp, w1bT, D, D, "w1b")
        A1B, A2 = [], []
        for m in range(KT):
            p1 = pmm([128, N])
            for k in range(KT):
                nc.tensor.matmul(p1[:], WA[k][:, 128 * m:128 * (m + 1)],
                                 ATT[k][:], start=(k == 0), stop=False)
            nc.tensor.matmul(p1[:], b1r[0:1, 128 * m:128 * (m + 1)],
                             ones_row[0:1, 0:N], start=False, stop=True)
            a1 = pRel.tile([128, N], F32R, tag=f"a1b{m}", name=f"a1b{m}")
            nc.scalar.copy(a1[:], p1[:])
            A1B.append(a1)
            p2 = pmm([128, N])
            for k in range(KT):
                nc.tensor.matmul(p2[:], WB[k][:, 128 * m:128 * (m + 1)],
                                 ATT[k][:], start=(k == 0), stop=(k == KT - 1))
            a2 = pRel.tile([128, N], F32R, tag=f"a2_{m}", name=f"a2_{m}")
            nc.scalar.copy(a2[:], p2[:])
            A2.append(a2)
        sW2.close()
        sAt.close()
        pG.close()
        sA.close()  # free phase-A psum

        # ---------------- phase B: relation MLP ----------------
        psB = ctx.enter_context(tc.tile_pool(name="psB", bufs=1, space="PSUM"))
        pB2 = ctx.enter_context(tc.tile_pool(name="pB2", bufs=1))

        for q in range(4):
            if fixed_q is not None and q != fixed_q:
                continue
            tiles = _segments(q)
            with (tc.If(qv == q) if fixed_q is None else contextlib.nullcontext()):
                for t in range(NTILES):
                    segs = tiles[t]
                    h1 = [pB2.tile([128, 512], F32R, tag=f"h1_{k}", bufs=2,
                                   name=f"h1_{k}") for k in range(KT)]
                    for k in range(KT):
                        for (ic, j0, ln, dst) in segs:
                            if k < 2:
                                nc.scalar.activation(
                                    h1[k][:, dst:dst + ln],
                                    A2[k][:, j0:j0 + ln], AF.Relu,
                                    bias=A1B[k][:, ic:ic + 1].bitcast(F32))
                            else:
                                eng = nc.vector if k < 4 else nc.gpsimd
                                eng.tensor_scalar(
                                    h1[k][:, dst:dst + ln],
                                    A2[k][:, j0:j0 + ln],
                                    A1B[k][:, ic:ic + 1].bitcast(F32), 0.0,
                                    OP.add, OP.max)
                    h2 = []
                    for m in range(3):
                        pm = psB.tile([128, 512], F32, tag="ph2", bufs=3,
                                      name="ph2")
                        for k in range(KT):
                            nc.tensor.matmul(pm[:],
                                             RW2[k][:, 128 * m:128 * (m + 1)],
                                             h1[k][:], start=(k == 0),
                                             stop=(k == KT - 1))
                        o = pB2.tile([128, 512], F32R, tag=f"h2_{m}", bufs=2,
                                     name=f"h2_{m}")
                        nc.scalar.activation(o[:], pm[:], AF.Relu,
                                             bias=rcb2[m])
                        h2.append(o)
                    pl = psB.tile([128, 32], F32, tag="plog", bufs=2, name="plog")
                    for pb in range(4):
                        for k in range(3):
                            nc.tensor.matmul(pl[:, 8 * pb:8 * (pb + 1)],
                                             h2[k][:, 128 * pb:128 * (pb + 1)],
                                             RW3[k][:], start=(k == 0),
                                             stop=False)
                        nc.tensor.matmul(pl[:, 8 * pb:8 * (pb + 1)],
                                         ones_row[0:1, 0:128], rcb3,
                                         start=False, stop=True)
                    ex = pB2.tile([128, 32], F32, tag="ex", bufs=2, name="ex")
                    nc.scalar.activation(ex[:], pl[:], AF.Exp)
                    sm = pB2.tile([128, 4], F32, tag="sm", bufs=2, name="sm")
                    nc.vector.tensor_reduce(
                        out=sm[:], in_=ex[:].rearrange("p (g x) -> p g x", g=4),
                        axis=mybir.AxisListType.X, op=OP.add)
                    rcp = pB2.tile([128, 4], F32, tag="rcp", bufs=2, name="rcp")
                    nc.vector.reciprocal(rcp[:], sm[:])
                    pr = pB2.tile([128, 32], F32, tag="pr", bufs=2, name="pr")
                    for g in range(4):
                        nc.vector.tensor_scalar_mul(pr[:, 8 * g:8 * (g + 1)],
                                                    ex[:, 8 * g:8 * (g + 1)],
                                                    rcp[:, g:g + 1])
                    # transpose [pairs, 8] -> [8, pairs] so the output DMA has
                    # 2KB-contiguous runs instead of 32B ones
                    prT = pB2.tile([8, 512], F32, tag="prT", bufs=2, name="prT")
                    for pb in range(4):
                        ptp = psB.tile([8, 128], F32, tag="ptp", bufs=2,
                                       name="ptp")
                        nc.tensor.transpose(
                            ptp[:], pr[:, 8 * pb:8 * (pb + 1)], ident[:])
                        nc.vector.tensor_copy(
                            prT[:, 128 * pb:128 * (pb + 1)], ptp[:])
                    nc.sync.dma_start(
                        rel_out[:, 512 * t:512 * (t + 1)], prT[:])
    nc.compile()
    return nc


def get_program():
    if "nc" not in _prog_cache:
        _prog_cache["nc"] = _build_program()
    return _prog_cache["nc"]


def make_in_maps(inputs):
    f = lambda a: np.ascontiguousarray(np.asarray(a), dtype=np.float32)
    x = f(inputs["node_embeddings"])
    ew = f(inputs["edge_weights"])
    ei = np.asarray(inputs["edge_indices"])
    src, tgt = ei[:, 0], ei[:, 1]
    ST = np.zeros((N, N), np.float32)
    np.add.at(ST, (src, tgt), ew)

    com = {"ST": ST}
    com["ones_in"] = np.ones((1, 512), np.float32)
    com["onesc_in"] = np.ones((128, 1), np.float32)
    com["oc_w1T"] = f(inputs["oc_w1"].T)
    com["oc_w2T"] = f(inputs["oc_w2"].T)
    com["oc_w3T"] = f(inputs["oc_w3"].T)
    com["ip_w1T"] = f(inputs["ip_w1"].T)
    com["ip_w2T"] = f(inputs["ip_w2"].T)
    com["ip_w3T"] = f(inputs["ip_w3"].T)
    for l in range(3):
        com[f"gnn{l}_w"] = np.concatenate(
            [f(inputs[f"gnn{l}_ws"].T), f(inputs[f"gnn{l}_wn"].T)], axis=1)
    com["attn_inT"] = f(inputs["attn_in_w"].T)
    com["attn_outT"] = f(inputs["attn_out_w"].T)
    rc_w1 = f(inputs["rc_w1"])
    com["w1aT"] = np.ascontiguousarray(rc_w1[:, :D].T)
    com["w1bT"] = np.ascontiguousarray(rc_w1[:, D:].T)
    com["rc_w2T"] = f(inputs["rc_w2"].T)
    com["rc_w3T"] = f(inputs["rc_w3"].T)

    def colpack(v, F):
        nk = (F + 127) // 128
        out = np.zeros((128, nk), np.float32)
        v = f(v).reshape(-1)
        for k in range(nk):
            p = min(128, F - 128 * k)
            out[:p, k] = v[128 * k:128 * k + p]
        return out

    bc = [colpack(inputs["oc_b1"], D), colpack(inputs["oc_b2"], 384),
          colpack(inputs["ip_b1"], 384), colpack(inputs["ip_b2"], 192)]
    for l in range(3):
        bsn = f(inputs[f"gnn{l}_bs"]) + f(inputs[f"gnn{l}_bn"])
        bc += [colpack(bsn, D), colpack(inputs[f"gnn{l}_g"], D),
               colpack(inputs[f"gnn{l}_beta"], D)]
    bc.append(colpack(inputs["rc_b2"], 384))
    com["bcols_in"] = np.concatenate(bc, axis=1)
    assert com["bcols_in"].shape == (128, 71)
    br = np.concatenate([
        f(inputs["oc_b3"]).reshape(-1), f(inputs["ip_b3"]).reshape(-1),
        f(inputs["attn_in_b"]).reshape(-1), f(inputs["attn_out_b"]).reshape(-1),
        f(inputs["rc_b1"]).reshape(-1), f(inputs["rc_b3"]).reshape(-1)])
    com["brows_in"] = br.reshape(1, -1)
    assert com["brows_in"].shape == (1, 3857)

    xT = [np.ascontiguousarray(x[b].T) for b in range(B)]
    in_maps = []
    for c in range(NCORES):
        m = dict(com)
        m["xT"] = xT[c // 4]
        in_maps.append(m)
    return in_maps


def assemble(results):
    obj = np.stack([results[0]["oc_out"], results[4]["oc_out"]])
    imp = np.stack([results[0]["ip_out"], results[4]["ip_out"]])
    att = np.stack([results[0]["att_out"], results[4]["att_out"]])
    attw = np.stack([results[0]["attw_out"], results[4]["attw_out"]])
    rel = np.zeros((B, P, 8), np.float32)
    base = lambda i: i * 255 - i * (i - 1) // 2
    for c in range(NCORES):
        b, q = c // 4, c % 4
        ro = results[c]["rel_out"].T
        for k, u in enumerate(_units(q)):
            s0 = 256 * k
            rel[b, base(u):base(u) + 255 - u] = ro[s0:s0 + 255 - u]
            rel[b, base(254 - u):base(254 - u) + u + 1] = ro[s0 + 255 - u:s0 + 256]
    return obj, rel, imp, att, attw


def kernel(**inputs):
    from concourse.bass_utils import run_bass_kernel_spmd
    nc = get_program()
    in_maps = make_in_maps(inputs)
    res = run_bass_kernel_spmd(nc, in_maps, list(range(NCORES))).results
    return assemble(res)
